# revision 29
# baseline (speedup 1.0000x reference)
"""Multi-head attention (B=2, S=4096, D=512, H=8) on 8 TRN2 NeuronCores.

Sharding: core = (batch, query-chunk-of-1024). Each core recomputes the K/V
projections for its batch (no collectives), runs attention over its query
chunk, and the output projection. Host splits/gathers.

v2: PE row-strip pairing. Score and PV matmuls alternate 64-row strips
(tile_position (0,0)/(64,0)) so consecutive matmuls execute concurrently in
disjoint PE row groups and LDWEIGHTS hides under the opposite strip's matmul
(~114ns/MM vs ~320ns serial). PV contraction is split into lo/hi 64-key
chains accumulating in separate PSUM tiles, merged during the normalize.
The softmax exp is split across engines: Scalar runs exact exp; Vector
computes a Schraudolph int16-bit-trick exp (one fused tensor_scalar op)
whose output is bitcast to fp16 for the PV matmul.
"""
import os
import sys

for _p in ("/opt/trn_rl_repo",):
    if _p not in sys.path:
        sys.path.insert(0, _p)

import numpy as np
from contextlib import ExitStack

import concourse.bass as bass
import concourse.bacc as bacc
import concourse.tile as tile
from concourse import mybir
from concourse.bass_utils import run_bass_kernel_spmd

F16 = mybir.dt.float16
F32 = mybir.dt.float32
I16 = mybir.dt.int16

D = 512          # d_model
H = 8            # heads
DK = 64          # head dim
SQ = 1024        # queries per core
SK = 4096        # keys (full batch sequence)
NCORES = 8
NCHUNK = SK // 128       # 128-key chunks (32)
NG = SK // 512           # 512-key kv dma groups (8)
CW = H * 65              # vh cols per chunk (8 heads x (64+ones))

# Schraudolph fp16-domain exp: bits = A*s + B (s = raw score, scale 1/8
# folded into A). Computed on DVE for a subset of tiles.
LOG2E = 1.4426950408889634
SCH_A = 0.125 * (2.0 ** 10) * LOG2E
SCH_B = 15.0 * (2.0 ** 10) - 44.6

# exp tile engine assignment: DVE takes tile when (idx % SCH_MOD) < SCH_CNT
SCH_MOD = int(os.environ.get("SCH_MOD", "4"))
SCH_CNT = int(os.environ.get("SCH_CNT", "1"))
PV_SPLIT = os.environ.get("PV_SPLIT", "1") == "1"   # strip-split PV chains
SC_PAIR = os.environ.get("SC_PAIR", "1") == "1"     # strip-paired SC

LAST_RESULTS = None      # BassKernelResults of the most recent run (for test.py)


def _build_kernel():
    nc = bacc.Bacc("TRN2", target_bir_lowering=False, debug=False,
                   num_devices=NCORES)

    qT = nc.dram_tensor("qT", [D, SQ], F16, kind="ExternalInput").ap()
    kT = nc.dram_tensor("kT", [D, SK], F16, kind="ExternalInput").ap()
    vT = nc.dram_tensor("vT", [D, SK], F16, kind="ExternalInput").ap()
    wq = nc.dram_tensor("wq", [D, D], F16, kind="ExternalInput").ap()
    wk = nc.dram_tensor("wk", [D, D], F16, kind="ExternalInput").ap()
    wv = nc.dram_tensor("wv", [D, D], F16, kind="ExternalInput").ap()
    wo = nc.dram_tensor("wo", [D, D], F16, kind="ExternalInput").ap()
    bq = nc.dram_tensor("bq", [D], F32, kind="ExternalInput").ap()
    bk = nc.dram_tensor("bk", [D], F32, kind="ExternalInput").ap()
    bv = nc.dram_tensor("bv", [D], F32, kind="ExternalInput").ap()
    bo = nc.dram_tensor("bo", [D], F32, kind="ExternalInput").ap()
    out = nc.dram_tensor("out", [SQ, D], F32, kind="ExternalOutput").ap()
    dbg = {}
    if os.environ.get("KDBG", "0") == "3":
        for nm, shp, dt in (("ssum", [65, 512], F32), ("rec", [1, 512], F32),
                            ("rep", [64, 512], F32), ("cats", [64, 512], F16)):
            dbg[nm] = nc.dram_tensor("d_" + nm, shp, dt,
                                     kind="ExternalOutput").ap()
    if os.environ.get("KDBG", "0") == "2":
        for nm, shp, dt in (("st0", [128, 512], F32), ("st1", [128, 512], F32),
                            ("pt0", [128, 512], F16), ("pt1", [128, 512], F16),
                            ("pva0", [128, 512], F32), ("pvb0", [128, 512], F32)):
            dbg[nm] = nc.dram_tensor("d_" + nm, shp, dt,
                                     kind="ExternalOutput").ap()
    if os.environ.get("KDBG", "0") == "1":
        dbg["qhT"] = nc.dram_tensor("d_qhT", [128, 4 * SQ], F16,
                                    kind="ExternalOutput").ap()
        dbg["khT"] = nc.dram_tensor("d_khT", [128, 4 * SK], F16,
                                    kind="ExternalOutput").ap()
        dbg["vh"] = nc.dram_tensor("d_vh", [128, NCHUNK * CW], F16,
                                   kind="ExternalOutput").ap()
        dbg["cat"] = nc.dram_tensor("d_cat", [128, 4 * SQ], F16,
                                    kind="ExternalOutput").ap()

    with tile.TileContext(nc) as tc:
        _emit(tc, qT, kT, vT, wq, wk, wv, wo, bq, bk, bv, bo, out, dbg)

    nc.compile()
    return nc


def _emit(tc, qT, kT, vT, wq, wk, wv, wo, bq, bk, bv, bo, out, dbg=None):
    nc = tc.nc
    Exp = mybir.ActivationFunctionType.Exp
    MULT = mybir.AluOpType.mult
    ADD = mybir.AluOpType.add

    with ExitStack() as ctx:
        const = ctx.enter_context(tc.tile_pool(name="const", bufs=1))
        kinp = ctx.enter_context(tc.tile_pool(name="kinp", bufs=1))
        vinp = ctx.enter_context(tc.tile_pool(name="vinp", bufs=3))
        pttp = ctx.enter_context(tc.tile_pool(name="pttp", bufs=5))
        schp = ctx.enter_context(tc.tile_pool(name="schp", bufs=3))
        normp = ctx.enter_context(tc.tile_pool(name="normp", bufs=2))
        outp = ctx.enter_context(tc.tile_pool(name="outp", bufs=2))
        opartp = ctx.enter_context(tc.tile_pool(name="opartp", bufs=1))
        # PSUM: st pool 4 banks (SC tiles + proj chains share the tag) +
        # pv lo/hi chains for 2 heads = 4 banks -> 8 total.
        stp = ctx.enter_context(tc.tile_pool(name="stp", bufs=2, space="PSUM"))
        pvp = ctx.enter_context(tc.tile_pool(
    name="pvp", bufs=(1 if PV_SPLIT else 2), space="PSUM"))

        # ---- persistent SBUF tensors -------------------------------------
        wq_sb = const.tile([128, 4 * D], F16)     # [dm%128, m*512 + d]
        wk_sb = const.tile([128, 4 * D], F16)
        wv_sb = const.tile([128, 4 * D], F16)
        wo_sb = const.tile([128, 4 * D], F16)     # [d%128, t*512 + e]
        qin = const.tile([128, 4 * SQ], F16)      # [dm%128, m*1024 + qq]
        qhT = const.tile([128, 4 * SQ], F16)      # [d%128, t*1024 + qq]
        khT = const.tile([128, 4 * SK], F16)      # [d%128, t*4096 + kk]
        vh = const.tile([128, NCHUNK * CW], F16)  # per chunk: h*65+dd, col 64=1
        cat = const.tile([128, 4 * SQ], F16)      # [d%128, t*1024 + qq]
        bqk = const.tile([128, 8], F32)           # cols 0-3 bq tiles, 4-7 bk
        bvo = const.tile([1, 2 * D], F32)         # bv | bo
        bv_rep = const.tile([128, D], F32)
        bo_rep = const.tile([128, D], F32)
        ones32 = const.tile([1, 128], F32)
        # resident kin: 8 groups x [128, 4*512]
        kin = [kinp.tile([128, 4 * 512], F16, tag=f"kin{g}", name=f"kin{g}")
               for g in range(NG)]

        # ---- startup loads: spread across 4 idle DMA queues -------------
        # sync: wq, bqk, then kin; gpsimd: qin; scalar: wk, bvo, wo;
        # vector: wv (vin comes later on gpsimd).
        for m in range(4):
            nc.sync.dma_start(wq_sb[:, m * D:(m + 1) * D], wq[m * 128:(m + 1) * 128, :])
            nc.gpsimd.dma_start(qin[:, m * SQ:m * SQ + 512], qT[m * 128:(m + 1) * 128, 0:512])
            nc.scalar.dma_start(wk_sb[:, m * D:(m + 1) * D], wk[m * 128:(m + 1) * 128, :])
            nc.gpsimd.dma_start(wv_sb[:, m * D:(m + 1) * D], wv[m * 128:(m + 1) * 128, :])
        nc.sync.dma_start(bqk[:, 0:4], bq.rearrange("(t p) -> p t", p=128))
        nc.sync.dma_start(bqk[:, 4:8], bk.rearrange("(t p) -> p t", p=128))
        for m in range(4):
            nc.gpsimd.dma_start(qin[:, m * SQ + 512:(m + 1) * SQ],
                                qT[m * 128:(m + 1) * 128, 512:SQ])
            nc.scalar.dma_start(wo_sb[:, m * D:(m + 1) * D], wo[m * 128:(m + 1) * 128, :])
        nc.scalar.dma_start(bvo[0:1, 0:D], bv.rearrange("(a d) -> a d", a=1))
        nc.scalar.dma_start(bvo[0:1, D:2 * D], bo.rearrange("(a d) -> a d", a=1))
        nc.vector.memset(ones32[:], 1.0)
        vh_ones = vh[:, :].rearrange("p (a c) -> p a c", c=65)[:, :, 64:65]
        nc.vector.memset(vh_ones, 1.0)
        # preload the exp table while startup DMAs run
        warm = normp.tile([1, 128], F16, tag="warm")
        nc.scalar.activation(warm[0:1, :], ones32[0:1, :], Exp)
        for src_off, rep in ((0, bv_rep), (D, bo_rep)):
            nc.gpsimd.partition_broadcast(rep[:, :], bvo[0:1, src_off:src_off + D])

        # kv input DMA: kin resident (all 8 groups), vin streamed
        def dma_kin(g, eng):
            for m in range(4):
                eng.dma_start(kin[g][:, m * 512:(m + 1) * 512],
                              kT[m * 128:(m + 1) * 128, g * 512:(g + 1) * 512])

        vin_tiles = {}

        def dma_vin(g, eng):
            t = vinp.tile([128, 4 * 512], F16, tag="vin")
            vin_tiles[g] = t
            for m in range(4):
                eng.dma_start(t[:, m * 512:(m + 1) * 512],
                              vT[m * 128:(m + 1) * 128, g * 512:(g + 1) * 512])

        # ---- projection chains (full-row serial; borrow "st" slots) ------
        def emit_qproj(p, qt):
            pjw = stp.tile([128, 1024], F32, tag="st", name="pjq")
            pj = pjw[:, 0:512]
            for m in range(4):
                nc.tensor.matmul(
                    pj, wq_sb[:, m * D + p * 128: m * D + (p + 1) * 128],
                    qin[:, m * SQ + qt * 512: m * SQ + qt * 512 + 512],
                    start=(m == 0), stop=(m == 3))
            nc.vector.tensor_scalar_add(
                qhT[:, p * SQ + qt * 512: p * SQ + qt * 512 + 512],
                pj, bqk[:, p:p + 1])

        def emit_kproj(t, g):
            pjw = stp.tile([128, 1024], F32, tag="st", name="pjk")
            pj = pjw[:, 0:512]
            for m in range(4):
                nc.tensor.matmul(
                    pj, wk_sb[:, m * D + t * 128: m * D + (t + 1) * 128],
                    kin[g][:, m * 512:(m + 1) * 512],
                    start=(m == 0), stop=(m == 3))
            nc.vector.tensor_scalar_add(
                khT[:, t * SK + g * 512: t * SK + g * 512 + 512],
                pj, bqk[:, 4 + t:5 + t])

        def emit_vproj(cc):
            g, ci = divmod(cc, 4)
            vin = vin_tiles[g]
            pjw = stp.tile([128, 1024], F32, tag="st", name="pjv")
            pj = pjw[:, 0:512]
            for m in range(4):
                nc.tensor.matmul(
                    pj, vin[:, m * 512 + ci * 128: m * 512 + ci * 128 + 128],
                    wv_sb[:, m * D:(m + 1) * D],
                    start=(m == 0), stop=(m == 3))
            dst = vh[:, cc * CW:(cc + 1) * CW]
            dst = dst.rearrange("p (h c) -> p h c", c=65)[:, :, 0:64]
            nc.vector.tensor_add(
                dst, pj.rearrange("p (h c) -> p h c", c=64),
                bv_rep[:].rearrange("p (h c) -> p h c", c=64))

        opart = [None] * 4

        def emit_oproj_partial(i):
            qb = 512 + i * 128
            pjw = stp.tile([128, 1024], F32, tag="st", name="pjp")
            pj = pjw[:, 0:512]
            for t in range(3):
                nc.tensor.matmul(
                    pj, cat[:, t * SQ + qb: t * SQ + qb + 128],
                    wo_sb[:, t * D:(t + 1) * D],
                    start=(t == 0), stop=(t == 2))
            acc = opartp.tile([128, 512], F32, tag=f"opart{i}", name="acc")
            nc.vector.scalar_tensor_tensor(
                acc[:], pj, 1.0, bo_rep[:], MULT, ADD)
            opart[i] = acc

        def emit_oproj_final(i):
            qb = 512 + i * 128
            pjw = stp.tile([128, 1024], F32, tag="st", name="pjf")
            pj = pjw[:, 0:512]
            nc.tensor.matmul(
                pj, cat[:, 3 * SQ + qb: 3 * SQ + qb + 128],
                wo_sb[:, 3 * D:4 * D], start=True, stop=True)
            ob = outp.tile([128, 512], F32, tag="ob")
            nc.vector.scalar_tensor_tensor(
                ob[:], pj, 1.0, opart[i][:], MULT, ADD)
            (nc.sync, nc.gpsimd)[i % 2].dma_start(out[qb:qb + 128, :], ob[:])

        def emit_oproj(qt, i):
            qb = qt * 512 + i * 128
            pjw = stp.tile([128, 1024], F32, tag="st", name="pjo")
            pj = pjw[:, 0:512]
            for t in range(4):
                nc.tensor.matmul(
                    pj, cat[:, t * SQ + qb: t * SQ + qb + 128],
                    wo_sb[:, t * D:(t + 1) * D],
                    start=(t == 0), stop=(t == 3))
            ob = outp.tile([128, 512], F32, tag="ob")
            nc.vector.tensor_add(ob[:], pj, bo_rep[:])
            (nc.sync, nc.gpsimd)[i % 2].dma_start(out[qb:qb + 128, :], ob[:])

        # ---- attention ---------------------------------------------------
        # sweep s: qt = s//4, pair p = s%4; heads h0=2p (strip rows 0:64 of
        # d-slice t=p), h1=2p+1 (rows 64:128).
        def sc_pair(s, c):
            qt, p = divmod(s, 4)
            qs = qt * 512
            stw = stp.tile([128, 1024], F32, tag="st", name="stw")
            nc.tensor.matmul(
                stw[:, 0:512],
                khT[0:64, p * SK + c * 128: p * SK + (c + 1) * 128],
                qhT[0:64, p * SQ + qs: p * SQ + qs + 512],
                tile_position=(0, 0))
            nc.tensor.matmul(
                stw[:, 512:1024],
                khT[64:128, p * SK + c * 128: p * SK + (c + 1) * 128],
                qhT[64:128, p * SQ + qs: p * SQ + qs + 512],
                tile_position=(64, 0))
            return stw

        def use_sch(s, c):
            if c >= 28:      # keep DVE free near sweep end (norm runs there)
                return False
            return ((s + 3 * c) % SCH_MOD) < SCH_CNT

        def emit_exp(s, c, stw):
            if use_sch(s, c):
                it = schp.tile([128, 1024], I16, tag="sch", name="sch")
                nc.vector.tensor_scalar(it[:], stw[:], SCH_A, SCH_B, MULT, ADD)
                w = it[:].bitcast(F16)
            else:
                pt = pttp.tile([128, 1024], F16, tag="pt", name="pt")
                nc.scalar.activation(pt[:], stw[:], Exp, scale=0.125)
                w = pt[:]
            return [w[:, 0:512], w[:, 512:1024]]

        def emit_pv(s, c, ptts, pv_tiles):
            qt, p = divmod(s, 4)
            for hi in (0, 1):
                h = 2 * p + hi
                pa, pb = pv_tiles[hi]
                ptt = ptts[hi]
                vcol = c * CW + h * 65
                if not PV_SPLIT:
                    nc.tensor.matmul(
                        pa[0:65, :], vh[:, vcol:vcol + 65], ptt[:, :],
                        start=(c == 0), stop=(c == NCHUNK - 1))
                    continue
                nc.tensor.matmul(
                    pa[0:65, :], vh[0:64, vcol:vcol + 65], ptt[0:64, :],
                    start=(c == 0), stop=(c == NCHUNK - 1),
                    tile_position=(0, 0))
                nc.tensor.matmul(
                    pb[0:65, :], vh[64:128, vcol:vcol + 65], ptt[64:128, :],
                    start=(c == 0), stop=(c == NCHUNK - 1),
                    tile_position=(64, 0))

        def emit_norm(s, hi, pv_tiles):
            qt, p = divmod(s, 4)
            h = 2 * p + hi
            pa, pb = pv_tiles[hi]
            ssum = normp.tile([65, 512], F32, tag="ssum")
            if PV_SPLIT:
                tmp = normp.tile([65, 512], F32, tag="tmp")
                nc.vector.tensor_copy(tmp[:], pa[0:65, :])
                nc.vector.tensor_add(ssum[:], tmp[:], pb[0:65, :])
            else:
                nc.vector.tensor_copy(ssum[:], pa[0:65, :])
            den = normp.tile([1, 512], F32, tag="den")
            nc.vector.tensor_copy(den[0:1, :], ssum[64:65, :])
            rec = normp.tile([1, 512], F32, tag="rec")
            nc.vector.reciprocal_approx_fast(rec[0:1, :], den[0:1, :])
            rep = normp.tile([64, 512], F32, tag="rep")
            nc.gpsimd.partition_broadcast(rep[:, :], rec[0:1, :])
            lo = hi * 64
            qs = qt * 512
            nc.vector.tensor_mul(
                cat[lo:lo + 64, p * SQ + qs: p * SQ + qs + 512],
                ssum[0:64, :], rep[:])
            if dbg and "ssum" in dbg and s == 7 and hi == 0:
                nc.sync.dma_start(dbg["ssum"], ssum[:])
                nc.sync.dma_start(dbg["rec"], rec[0:1, :])
                nc.sync.dma_start(dbg["rep"], rep[:])
                nc.sync.dma_start(
                    dbg["cats"],
                    cat[lo:lo + 64, p * SQ + qs: p * SQ + qs + 512])

        # ---- main schedule ----------------------------------------------
        # startup: kin groups 0-2, vin groups 0-1, qproj(0,0)
        dma_kin(0, nc.sync)
        dma_vin(0, nc.gpsimd)
        dma_kin(1, nc.sync)
        dma_vin(1, nc.gpsimd)
        dma_kin(2, nc.sync)
        emit_qproj(0, 0)
        emit_kproj(0, 0)

        for s in range(8):
            qt, p = divmod(s, 4)
            st_hist = {}
            ptt_hist = {}
            if PV_SPLIT:
                pv_tiles = {
                    0: (pvp.tile([128, 512], F32, tag="pva0", name="pva0"),
                        pvp.tile([128, 512], F32, tag="pvb0", name="pvb0")),
                    1: (pvp.tile([128, 512], F32, tag="pva1", name="pva1"),
                        pvp.tile([128, 512], F32, tag="pvb1", name="pvb1")),
                }
            else:
                pv_tiles = {
                    0: (pvp.tile([128, 512], F32, tag="pva0", name="pva0"),
                        None),
                    1: (pvp.tile([128, 512], F32, tag="pva1", name="pva1"),
                        None),
                }
            for c in range(NCHUNK + 2):
                if c < NCHUNK:
                    # kv pipeline work. kproj(t, g) must complete before
                    # chunk 4g of sweep t; group 0 is pre-issued (startup or
                    # previous sweep's drain), groups g>=1 at chunk 4(g-1).
                    if s == 0:
                        if c % 4 == 0 and c // 4 + 2 < NG:
                            dma_vin(c // 4 + 2, nc.gpsimd)
                        if c % 4 == 1 and c // 4 + 3 < NG:
                            dma_kin(c // 4 + 3, nc.sync)
                        if c < 2:
                            emit_vproj(c)        # lag-2: due before PV(c)
                        if c + 2 < NCHUNK:
                            emit_vproj(c + 2)
                        if c % 4 == 0 and c // 4 + 1 < NG:
                            emit_kproj(0, c // 4 + 1)
                    elif s in (1, 2, 3):
                        if c % 4 == 0 and c // 4 + 1 < NG:
                            emit_kproj(s, c // 4 + 1)
                    st_hist[c] = sc_pair(s, c)
                if c >= 1 and c - 1 < NCHUNK:
                    ptt_hist[c - 1] = emit_exp(s, c - 1, st_hist.pop(c - 1))
                if c >= 2:
                    emit_pv(s, c - 2, ptt_hist.pop(c - 2), pv_tiles)
                # spread next sweep's qproj + oproj into the chunk loop
                if c == 16 and s < 7:
                    nqt, np_ = divmod(s + 1, 4)
                    emit_qproj(np_, nqt)
                if c == NCHUNK and s in (0, 1, 2):
                    emit_kproj(s + 1, 0)     # next sweep's group 0
                if c == 24 and s >= 4:
                    emit_oproj(0, s - 4)
                if s == 7 and c in (6, 11, 16, 21):
                    emit_oproj_partial((c - 6) // 5)
            for hi in (0, 1):
                emit_norm(s, hi, pv_tiles)
            if os.environ.get("NORM_BARRIER", "0") == "1":
                tc.strict_bb_all_engine_barrier()

        for i in range(4):
            emit_oproj_final(i)

        if dbg and "pva0" in dbg:
            for nm, tl in (("pva0", pv_tiles[0][0]), ("pvb0", pv_tiles[0][1])):
                dcq = outp.tile([128, 512], F32, tag="d" + nm, name="dcq")
                nc.vector.tensor_copy(dcq[:], tl[:])
                nc.sync.dma_start(dbg[nm], dcq[:])
        if dbg and "qhT" in dbg:
            nc.sync.dma_start(dbg["qhT"], qhT[:])
            nc.sync.dma_start(dbg["khT"], khT[:])
            nc.sync.dma_start(dbg["vh"], vh[:])
            nc.sync.dma_start(dbg["cat"], cat[:])


_NC_CACHE = None


def _get_nc():
    global _NC_CACHE
    if _NC_CACHE is None:
        _NC_CACHE = _build_kernel()
    return _NC_CACHE


def _spot_check(full, q, k, v, Wq, bq, Wk, bk, Wv, bv, Wo, bo):
    """Verify a few rows of the output against numpy. Catches the rare
    bad-schedule compile (timing race) so the caller can rebuild + retry."""
    rng = np.random.default_rng(12345)
    rows = [(int(b), int(r)) for b, r in
            zip(rng.integers(0, 2, 4), rng.integers(0, SK, 4))]
    for b, r in rows:
        qh = (q[b, r:r + 1] @ Wq.T + bq).reshape(H, DK)
        kh = (k[b] @ Wk.T + bk).reshape(SK, H, DK)
        vh = (v[b] @ Wv.T + bv).reshape(SK, H, DK)
        outh = np.zeros((H, DK), np.float32)
        for h in range(H):
            s = (kh[:, h] @ qh[h]) / 8.0
            s -= s.max()
            p = np.exp(s)
            outh[h] = (p @ vh[:, h]) / p.sum()
        exp_row = outh.reshape(D) @ Wo.T + bo
        got = full[b, r]
        rel = np.linalg.norm(got - exp_row) / (np.linalg.norm(exp_row) + 1e-9)
        if not np.isfinite(got).all() or rel > 0.05:
            return False
    return True


def kernel(q, k, v, Wq, bq, Wk, bk, Wv, bv, Wo, bo, trace=False):
    global LAST_RESULTS, _NC_CACHE
    q = np.asarray(q, np.float32)
    k = np.asarray(k, np.float32)
    v = np.asarray(v, np.float32)

    kT16 = [np.ascontiguousarray(k[b].T).astype(np.float16) for b in range(2)]
    vT16 = [np.ascontiguousarray(v[b].T).astype(np.float16) for b in range(2)]
    wq16 = np.ascontiguousarray(np.asarray(Wq, np.float32).T).astype(np.float16)
    wk16 = np.ascontiguousarray(np.asarray(Wk, np.float32).T).astype(np.float16)
    wv16 = np.ascontiguousarray(np.asarray(Wv, np.float32).T).astype(np.float16)
    wo16 = np.ascontiguousarray(np.asarray(Wo, np.float32).T).astype(np.float16)
    bq32 = np.ascontiguousarray(np.asarray(bq, np.float32))
    bk32 = np.ascontiguousarray(np.asarray(bk, np.float32))
    bv32 = np.ascontiguousarray(np.asarray(bv, np.float32))
    bo32 = np.ascontiguousarray(np.asarray(bo, np.float32))

    in_maps = []
    for core in range(NCORES):
        b, c = divmod(core, 4)
        qT16 = np.ascontiguousarray(
            q[b, c * SQ:(c + 1) * SQ, :].T).astype(np.float16)
        in_maps.append({
            "qT": qT16, "kT": kT16[b], "vT": vT16[b],
            "wq": wq16, "wk": wk16, "wv": wv16, "wo": wo16,
            "bq": bq32, "bk": bk32, "bv": bv32, "bo": bo32,
        })

    Wq32 = np.asarray(Wq, np.float32)
    Wk32 = np.asarray(Wk, np.float32)
    Wv32 = np.asarray(Wv, np.float32)
    Wo32 = np.asarray(Wo, np.float32)

    for attempt in range(3):
        nc = _get_nc()
        res = run_bass_kernel_spmd(nc, in_maps, core_ids=list(range(NCORES)),
                                   trace=trace)
        LAST_RESULTS = res

        full = np.empty((2, SK, D), np.float32)
        for core in range(NCORES):
            b, c = divmod(core, 4)
            full[b, c * SQ:(c + 1) * SQ, :] = res.results[core]["out"]
        if _spot_check(full, q, k, v, Wq32, bq32, Wk32, bk32,
                       Wv32, bv32, Wo32, bo32):
            return full
        # bad-schedule compile or transient failure: rebuild and retry
        _NC_CACHE = None
    return full


# revision 30
# speedup vs baseline: 1.0687x; 1.0687x over previous
"""Multi-head attention (B=2, S=4096, D=512, H=8) on 8 TRN2 NeuronCores.

Sharding: core = (batch, query-chunk-of-1024). Each core recomputes the K/V
projections for its batch (no collectives), runs attention over its query
chunk, and the output projection. Host splits/gathers.

v2: PE row-strip pairing. Score and PV matmuls alternate 64-row strips
(tile_position (0,0)/(64,0)) so consecutive matmuls execute concurrently in
disjoint PE row groups and LDWEIGHTS hides under the opposite strip's matmul
(~114ns/MM vs ~320ns serial). PV contraction is split into lo/hi 64-key
chains accumulating in separate PSUM tiles, merged during the normalize.
The softmax exp is split across engines: Scalar runs exact exp; Vector
computes a Schraudolph int16-bit-trick exp (one fused tensor_scalar op)
whose output is bitcast to fp16 for the PV matmul.
"""
import os
import sys

for _p in ("/opt/trn_rl_repo",):
    if _p not in sys.path:
        sys.path.insert(0, _p)

import numpy as np
from contextlib import ExitStack

import concourse.bass as bass
import concourse.bacc as bacc
import concourse.tile as tile
from concourse import mybir
from concourse.bass_utils import run_bass_kernel_spmd

F16 = mybir.dt.float16
F32 = mybir.dt.float32
I16 = mybir.dt.int16

D = 512          # d_model
H = 8            # heads
DK = 64          # head dim
SQ = 1024        # queries per core
SK = 4096        # keys (full batch sequence)
NCORES = 8
NCHUNK = SK // 128       # 128-key chunks (32)
NG = SK // 512           # 512-key kv dma groups (8)
CW = H * 65              # vh cols per chunk (8 heads x (64+ones))

# Schraudolph fp16-domain exp: bits = A*s + B (s = raw score, scale 1/8
# folded into A). Computed on DVE for a subset of tiles.
LOG2E = 1.4426950408889634
SCH_A = 0.125 * (2.0 ** 10) * LOG2E
SCH_B = 15.0 * (2.0 ** 10) - 44.6

# exp tile engine assignment: DVE takes tile when (idx % SCH_MOD) < SCH_CNT
SCH_MOD = int(os.environ.get("SCH_MOD", "4"))
SCH_CNT = int(os.environ.get("SCH_CNT", "1"))
PV_SPLIT = os.environ.get("PV_SPLIT", "1") == "1"   # strip-split PV chains
SC_PAIR = os.environ.get("SC_PAIR", "1") == "1"     # strip-paired SC

LAST_RESULTS = None      # BassKernelResults of the most recent run (for test.py)


def _build_kernel():
    nc = bacc.Bacc("TRN2", target_bir_lowering=False, debug=False,
                   num_devices=NCORES)

    qT = nc.dram_tensor("qT", [D, SQ], F16, kind="ExternalInput").ap()
    kT = nc.dram_tensor("kT", [D, SK], F16, kind="ExternalInput").ap()
    vT = nc.dram_tensor("vT", [D, SK], F16, kind="ExternalInput").ap()
    wq = nc.dram_tensor("wq", [D, D], F16, kind="ExternalInput").ap()
    wk = nc.dram_tensor("wk", [D, D], F16, kind="ExternalInput").ap()
    wv = nc.dram_tensor("wv", [D, D], F16, kind="ExternalInput").ap()
    wo = nc.dram_tensor("wo", [D, D], F16, kind="ExternalInput").ap()
    bq = nc.dram_tensor("bq", [D], F32, kind="ExternalInput").ap()
    bk = nc.dram_tensor("bk", [D], F32, kind="ExternalInput").ap()
    bv = nc.dram_tensor("bv", [D], F32, kind="ExternalInput").ap()
    bo = nc.dram_tensor("bo", [D], F32, kind="ExternalInput").ap()
    out = nc.dram_tensor("out", [SQ, D], F32, kind="ExternalOutput").ap()
    dbg = {}
    if os.environ.get("KDBG", "0") == "3":
        for nm, shp, dt in (("ssum", [65, 512], F32), ("rec", [1, 512], F32),
                            ("rep", [64, 512], F32), ("cats", [64, 512], F16)):
            dbg[nm] = nc.dram_tensor("d_" + nm, shp, dt,
                                     kind="ExternalOutput").ap()
    if os.environ.get("KDBG", "0") == "2":
        for nm, shp, dt in (("st0", [128, 512], F32), ("st1", [128, 512], F32),
                            ("pt0", [128, 512], F16), ("pt1", [128, 512], F16),
                            ("pva0", [128, 512], F32), ("pvb0", [128, 512], F32)):
            dbg[nm] = nc.dram_tensor("d_" + nm, shp, dt,
                                     kind="ExternalOutput").ap()
    if os.environ.get("KDBG", "0") == "1":
        dbg["qhT"] = nc.dram_tensor("d_qhT", [128, 4 * SQ], F16,
                                    kind="ExternalOutput").ap()
        dbg["khT"] = nc.dram_tensor("d_khT", [128, 4 * SK], F16,
                                    kind="ExternalOutput").ap()
        dbg["vh"] = nc.dram_tensor("d_vh", [128, NCHUNK * CW], F16,
                                   kind="ExternalOutput").ap()
        dbg["cat"] = nc.dram_tensor("d_cat", [128, 4 * SQ], F16,
                                    kind="ExternalOutput").ap()

    with tile.TileContext(nc) as tc:
        _emit(tc, qT, kT, vT, wq, wk, wv, wo, bq, bk, bv, bo, out, dbg)

    nc.compile()
    return nc


def _emit(tc, qT, kT, vT, wq, wk, wv, wo, bq, bk, bv, bo, out, dbg=None):
    nc = tc.nc
    Exp = mybir.ActivationFunctionType.Exp
    MULT = mybir.AluOpType.mult
    ADD = mybir.AluOpType.add

    with ExitStack() as ctx:
        const = ctx.enter_context(tc.tile_pool(name="const", bufs=1))
        kinp = ctx.enter_context(tc.tile_pool(name="kinp", bufs=1))
        vinp = ctx.enter_context(tc.tile_pool(name="vinp", bufs=3))
        pttp = ctx.enter_context(tc.tile_pool(name="pttp", bufs=10))
        schp = ctx.enter_context(tc.tile_pool(name="schp", bufs=10))
        normp = ctx.enter_context(tc.tile_pool(name="normp", bufs=2))
        outp = ctx.enter_context(tc.tile_pool(name="outp", bufs=2))
        opartp = ctx.enter_context(tc.tile_pool(name="opartp", bufs=1))
        # PSUM: st pool 4 banks (SC tiles + proj chains share the tag) +
        # pv lo/hi chains for 2 heads = 4 banks -> 8 total.
        stp = ctx.enter_context(tc.tile_pool(name="stp", bufs=4, space="PSUM"))
        pvp = ctx.enter_context(tc.tile_pool(
    name="pvp", bufs=(1 if PV_SPLIT else 2), space="PSUM"))

        # ---- persistent SBUF tensors -------------------------------------
        wq_sb = const.tile([128, 4 * D], F16)     # [dm%128, m*512 + d]
        wk_sb = const.tile([128, 4 * D], F16)
        wv_sb = const.tile([128, 4 * D], F16)
        wo_sb = const.tile([128, 4 * D], F16)     # [d%128, t*512 + e]
        qin = const.tile([128, 4 * SQ], F16)      # [dm%128, m*1024 + qq]
        qhT = const.tile([128, 4 * SQ], F16)      # [d%128, t*1024 + qq]
        khT = const.tile([128, 4 * SK], F16)      # [d%128, t*4096 + kk]
        vh = const.tile([128, NCHUNK * CW], F16)  # per chunk: h*65+dd, col 64=1
        cat = const.tile([128, 4 * SQ], F16)      # [d%128, t*1024 + qq]
        bqk = const.tile([128, 8], F32)           # cols 0-3 bq tiles, 4-7 bk
        bvo = const.tile([1, 2 * D], F32)         # bv | bo
        bv_rep = const.tile([128, D], F32)
        bo_rep = const.tile([128, D], F32)
        ones32 = const.tile([1, 128], F32)
        # resident kin: 8 groups x [128, 4*512]
        kin = [kinp.tile([128, 4 * 512], F16, tag=f"kin{g}", name=f"kin{g}")
               for g in range(NG)]

        # ---- startup loads: spread across 4 idle DMA queues -------------
        # sync: wq, bqk, then kin; gpsimd: qin; scalar: wk, bvo, wo;
        # vector: wv (vin comes later on gpsimd).
        for m in range(4):
            nc.sync.dma_start(wq_sb[:, m * D:(m + 1) * D], wq[m * 128:(m + 1) * 128, :])
            nc.gpsimd.dma_start(qin[:, m * SQ:m * SQ + 512], qT[m * 128:(m + 1) * 128, 0:512])
            nc.scalar.dma_start(wk_sb[:, m * D:(m + 1) * D], wk[m * 128:(m + 1) * 128, :])
            nc.gpsimd.dma_start(wv_sb[:, m * D:(m + 1) * D], wv[m * 128:(m + 1) * 128, :])
        nc.sync.dma_start(bqk[:, 0:4], bq.rearrange("(t p) -> p t", p=128))
        nc.sync.dma_start(bqk[:, 4:8], bk.rearrange("(t p) -> p t", p=128))
        for m in range(4):
            nc.gpsimd.dma_start(qin[:, m * SQ + 512:(m + 1) * SQ],
                                qT[m * 128:(m + 1) * 128, 512:SQ])
            nc.scalar.dma_start(wo_sb[:, m * D:(m + 1) * D], wo[m * 128:(m + 1) * 128, :])
        nc.scalar.dma_start(bvo[0:1, 0:D], bv.rearrange("(a d) -> a d", a=1))
        nc.scalar.dma_start(bvo[0:1, D:2 * D], bo.rearrange("(a d) -> a d", a=1))
        nc.vector.memset(ones32[:], 1.0)
        vh_ones = vh[:, :].rearrange("p (a c) -> p a c", c=65)[:, :, 64:65]
        nc.vector.memset(vh_ones, 1.0)
        # preload the exp table while startup DMAs run
        warm = normp.tile([1, 128], F16, tag="warm")
        nc.scalar.activation(warm[0:1, :], ones32[0:1, :], Exp)
        for src_off, rep in ((0, bv_rep), (D, bo_rep)):
            nc.gpsimd.partition_broadcast(rep[:, :], bvo[0:1, src_off:src_off + D])

        # kv input DMA: kin resident (all 8 groups), vin streamed
        def dma_kin(g, eng):
            for m in range(4):
                eng.dma_start(kin[g][:, m * 512:(m + 1) * 512],
                              kT[m * 128:(m + 1) * 128, g * 512:(g + 1) * 512])

        vin_tiles = {}

        def dma_vin(g, eng):
            t = vinp.tile([128, 4 * 512], F16, tag="vin")
            vin_tiles[g] = t
            for m in range(4):
                eng.dma_start(t[:, m * 512:(m + 1) * 512],
                              vT[m * 128:(m + 1) * 128, g * 512:(g + 1) * 512])

        # ---- projection chains (full-row serial; borrow "st" slots) ------
        def emit_qproj(p, qt):
            pj_t = stp.tile([128, 512], F32, tag="st", name="pjq")
            pj = pj_t[:]
            for m in range(4):
                nc.tensor.matmul(
                    pj, wq_sb[:, m * D + p * 128: m * D + (p + 1) * 128],
                    qin[:, m * SQ + qt * 512: m * SQ + qt * 512 + 512],
                    start=(m == 0), stop=(m == 3))
            nc.vector.tensor_scalar_add(
                qhT[:, p * SQ + qt * 512: p * SQ + qt * 512 + 512],
                pj, bqk[:, p:p + 1])

        def emit_kproj(t, g):
            pj_t = stp.tile([128, 512], F32, tag="st", name="pjk")
            pj = pj_t[:]
            for m in range(4):
                nc.tensor.matmul(
                    pj, wk_sb[:, m * D + t * 128: m * D + (t + 1) * 128],
                    kin[g][:, m * 512:(m + 1) * 512],
                    start=(m == 0), stop=(m == 3))
            nc.vector.tensor_scalar_add(
                khT[:, t * SK + g * 512: t * SK + g * 512 + 512],
                pj, bqk[:, 4 + t:5 + t])

        def emit_vproj(cc):
            g, ci = divmod(cc, 4)
            vin = vin_tiles[g]
            pj_t = stp.tile([128, 512], F32, tag="st", name="pjv")
            pj = pj_t[:]
            for m in range(4):
                nc.tensor.matmul(
                    pj, vin[:, m * 512 + ci * 128: m * 512 + ci * 128 + 128],
                    wv_sb[:, m * D:(m + 1) * D],
                    start=(m == 0), stop=(m == 3))
            dst = vh[:, cc * CW:(cc + 1) * CW]
            dst = dst.rearrange("p (h c) -> p h c", c=65)[:, :, 0:64]
            nc.vector.tensor_add(
                dst, pj.rearrange("p (h c) -> p h c", c=64),
                bv_rep[:].rearrange("p (h c) -> p h c", c=64))

        opart = [None] * 4

        def emit_oproj_partial(i):
            qb = 512 + i * 128
            pj_t = stp.tile([128, 512], F32, tag="st", name="pjp")
            pj = pj_t[:]
            for t in range(3):
                nc.tensor.matmul(
                    pj, cat[:, t * SQ + qb: t * SQ + qb + 128],
                    wo_sb[:, t * D:(t + 1) * D],
                    start=(t == 0), stop=(t == 2))
            acc = opartp.tile([128, 512], F32, tag=f"opart{i}", name="acc")
            nc.vector.scalar_tensor_tensor(
                acc[:], pj, 1.0, bo_rep[:], MULT, ADD)
            opart[i] = acc

        def emit_oproj_final(i):
            qb = 512 + i * 128
            pj_t = stp.tile([128, 512], F32, tag="st", name="pjf")
            pj = pj_t[:]
            nc.tensor.matmul(
                pj, cat[:, 3 * SQ + qb: 3 * SQ + qb + 128],
                wo_sb[:, 3 * D:4 * D], start=True, stop=True)
            ob = outp.tile([128, 512], F32, tag="ob")
            nc.vector.scalar_tensor_tensor(
                ob[:], pj, 1.0, opart[i][:], MULT, ADD)
            (nc.sync, nc.gpsimd)[i % 2].dma_start(out[qb:qb + 128, :], ob[:])

        def emit_oproj(qt, i):
            qb = qt * 512 + i * 128
            pj_t = stp.tile([128, 512], F32, tag="st", name="pjo")
            pj = pj_t[:]
            for t in range(4):
                nc.tensor.matmul(
                    pj, cat[:, t * SQ + qb: t * SQ + qb + 128],
                    wo_sb[:, t * D:(t + 1) * D],
                    start=(t == 0), stop=(t == 3))
            ob = outp.tile([128, 512], F32, tag="ob")
            nc.vector.tensor_add(ob[:], pj, bo_rep[:])
            (nc.sync, nc.gpsimd)[i % 2].dma_start(out[qb:qb + 128, :], ob[:])

        # ---- attention ---------------------------------------------------
        # sweep s: qt = s//4, pair p = s%4; heads h0=2p (strip rows 0:64 of
        # d-slice t=p), h1=2p+1 (rows 64:128).
        def sc_pair(s, c):
            qt, p = divmod(s, 4)
            qs = qt * 512
            st0 = stp.tile([128, 512], F32, tag="st", name="st0")
            st1 = stp.tile([128, 512], F32, tag="st", name="st1")
            nc.tensor.matmul(
                st0[:], khT[0:64, p * SK + c * 128: p * SK + (c + 1) * 128],
                qhT[0:64, p * SQ + qs: p * SQ + qs + 512],
                tile_position=(0, 0))
            nc.tensor.matmul(
                st1[:], khT[64:128, p * SK + c * 128: p * SK + (c + 1) * 128],
                qhT[64:128, p * SQ + qs: p * SQ + qs + 512],
                tile_position=(64, 0))
            return st0, st1

        def use_sch(s, c, hi):
            if c >= 28:      # keep DVE free near sweep end (norm runs there)
                return False
            return ((s + 2 * c + hi) % SCH_MOD) < SCH_CNT

        def emit_exp(s, c, st_pair):
            ptts = []
            for hi in (0, 1):
                st = st_pair[hi]
                if use_sch(s, c, hi):
                    it = schp.tile([128, 512], I16, tag="sch", name="sch")
                    nc.vector.tensor_scalar(it[:], st[:], SCH_A, SCH_B, MULT, ADD)
                    ptts.append(it[:].bitcast(F16))
                else:
                    pt = pttp.tile([128, 512], F16, tag="pt", name="pt")
                    nc.scalar.activation(pt[:], st[:], Exp, scale=0.125)
                    ptts.append(pt[:])
            return ptts

        def emit_pv(s, c, ptts, pv_tiles):
            qt, p = divmod(s, 4)
            for hi in (0, 1):
                h = 2 * p + hi
                pa, pb = pv_tiles[hi]
                ptt = ptts[hi]
                vcol = c * CW + h * 65
                if not PV_SPLIT:
                    nc.tensor.matmul(
                        pa[0:65, :], vh[:, vcol:vcol + 65], ptt[:, :],
                        start=(c == 0), stop=(c == NCHUNK - 1))
                    continue
                nc.tensor.matmul(
                    pa[0:65, :], vh[0:64, vcol:vcol + 65], ptt[0:64, :],
                    start=(c == 0), stop=(c == NCHUNK - 1),
                    tile_position=(0, 0))
                nc.tensor.matmul(
                    pb[0:65, :], vh[64:128, vcol:vcol + 65], ptt[64:128, :],
                    start=(c == 0), stop=(c == NCHUNK - 1),
                    tile_position=(64, 0))

        def emit_norm(s, hi, pv_tiles):
            qt, p = divmod(s, 4)
            h = 2 * p + hi
            pa, pb = pv_tiles[hi]
            ssum = normp.tile([65, 512], F32, tag="ssum")
            if PV_SPLIT:
                tmp = normp.tile([65, 512], F32, tag="tmp")
                nc.vector.tensor_copy(tmp[:], pa[0:65, :])
                nc.vector.tensor_add(ssum[:], tmp[:], pb[0:65, :])
            else:
                nc.vector.tensor_copy(ssum[:], pa[0:65, :])
            den = normp.tile([1, 512], F32, tag="den")
            nc.vector.tensor_copy(den[0:1, :], ssum[64:65, :])
            rec = normp.tile([1, 512], F32, tag="rec")
            nc.vector.reciprocal_approx_fast(rec[0:1, :], den[0:1, :])
            rep = normp.tile([64, 512], F32, tag="rep")
            nc.gpsimd.partition_broadcast(rep[:, :], rec[0:1, :])
            lo = hi * 64
            qs = qt * 512
            nc.vector.tensor_mul(
                cat[lo:lo + 64, p * SQ + qs: p * SQ + qs + 512],
                ssum[0:64, :], rep[:])
            if dbg and "ssum" in dbg and s == 7 and hi == 0:
                nc.sync.dma_start(dbg["ssum"], ssum[:])
                nc.sync.dma_start(dbg["rec"], rec[0:1, :])
                nc.sync.dma_start(dbg["rep"], rep[:])
                nc.sync.dma_start(
                    dbg["cats"],
                    cat[lo:lo + 64, p * SQ + qs: p * SQ + qs + 512])

        # ---- main schedule ----------------------------------------------
        # startup: kin groups 0-2, vin groups 0-1, qproj(0,0)
        dma_kin(0, nc.sync)
        dma_vin(0, nc.gpsimd)
        dma_kin(1, nc.sync)
        dma_vin(1, nc.gpsimd)
        dma_kin(2, nc.sync)
        emit_qproj(0, 0)
        emit_kproj(0, 0)

        for s in range(8):
            qt, p = divmod(s, 4)
            st_hist = {}
            ptt_hist = {}
            if PV_SPLIT:
                pv_tiles = {
                    0: (pvp.tile([128, 512], F32, tag="pva0", name="pva0"),
                        pvp.tile([128, 512], F32, tag="pvb0", name="pvb0")),
                    1: (pvp.tile([128, 512], F32, tag="pva1", name="pva1"),
                        pvp.tile([128, 512], F32, tag="pvb1", name="pvb1")),
                }
            else:
                pv_tiles = {
                    0: (pvp.tile([128, 512], F32, tag="pva0", name="pva0"),
                        None),
                    1: (pvp.tile([128, 512], F32, tag="pva1", name="pva1"),
                        None),
                }
            for c in range(NCHUNK + 2):
                if c < NCHUNK:
                    # kv pipeline work. kproj(t, g) must complete before
                    # chunk 4g of sweep t; group 0 is pre-issued (startup or
                    # previous sweep's drain), groups g>=1 at chunk 4(g-1).
                    if s == 0:
                        if c % 4 == 0 and c // 4 + 2 < NG:
                            dma_vin(c // 4 + 2, nc.gpsimd)
                        if c % 4 == 1 and c // 4 + 3 < NG:
                            dma_kin(c // 4 + 3, nc.sync)
                        if c < 2:
                            emit_vproj(c)        # lag-2: due before PV(c)
                        if c + 2 < NCHUNK:
                            emit_vproj(c + 2)
                        if c % 4 == 0 and c // 4 + 1 < NG:
                            emit_kproj(0, c // 4 + 1)
                    elif s in (1, 2, 3):
                        if c % 4 == 0 and c // 4 + 1 < NG:
                            emit_kproj(s, c // 4 + 1)
                    st_hist[c] = sc_pair(s, c)
                if c >= 1 and c - 1 < NCHUNK:
                    ptt_hist[c - 1] = emit_exp(s, c - 1, st_hist.pop(c - 1))
                if c >= 2:
                    emit_pv(s, c - 2, ptt_hist.pop(c - 2), pv_tiles)
                # spread next sweep's qproj + oproj into the chunk loop
                if c == 16 and s < 7:
                    nqt, np_ = divmod(s + 1, 4)
                    emit_qproj(np_, nqt)
                if c == NCHUNK and s in (0, 1, 2):
                    emit_kproj(s + 1, 0)     # next sweep's group 0
                if c == 24 and s >= 4:
                    emit_oproj(0, s - 4)
            for hi in (0, 1):
                emit_norm(s, hi, pv_tiles)
            if os.environ.get("NORM_BARRIER", "0") == "1":
                tc.strict_bb_all_engine_barrier()

        for i in range(4):
            emit_oproj(1, i)

        if dbg and "pva0" in dbg:
            for nm, tl in (("pva0", pv_tiles[0][0]), ("pvb0", pv_tiles[0][1])):
                dcq = outp.tile([128, 512], F32, tag="d" + nm, name="dcq")
                nc.vector.tensor_copy(dcq[:], tl[:])
                nc.sync.dma_start(dbg[nm], dcq[:])
        if dbg and "qhT" in dbg:
            nc.sync.dma_start(dbg["qhT"], qhT[:])
            nc.sync.dma_start(dbg["khT"], khT[:])
            nc.sync.dma_start(dbg["vh"], vh[:])
            nc.sync.dma_start(dbg["cat"], cat[:])


_NC_CACHE = None


def _get_nc():
    global _NC_CACHE
    if _NC_CACHE is None:
        _NC_CACHE = _build_kernel()
    return _NC_CACHE


def _spot_check(full, q, k, v, Wq, bq, Wk, bk, Wv, bv, Wo, bo):
    """Verify a few rows of the output against numpy. Catches the rare
    bad-schedule compile (timing race) so the caller can rebuild + retry."""
    rng = np.random.default_rng(12345)
    rows = [(int(b), int(r)) for b, r in
            zip(rng.integers(0, 2, 4), rng.integers(0, SK, 4))]
    for b, r in rows:
        qh = (q[b, r:r + 1] @ Wq.T + bq).reshape(H, DK)
        kh = (k[b] @ Wk.T + bk).reshape(SK, H, DK)
        vh = (v[b] @ Wv.T + bv).reshape(SK, H, DK)
        outh = np.zeros((H, DK), np.float32)
        for h in range(H):
            s = (kh[:, h] @ qh[h]) / 8.0
            s -= s.max()
            p = np.exp(s)
            outh[h] = (p @ vh[:, h]) / p.sum()
        exp_row = outh.reshape(D) @ Wo.T + bo
        got = full[b, r]
        rel = np.linalg.norm(got - exp_row) / (np.linalg.norm(exp_row) + 1e-9)
        if not np.isfinite(got).all() or rel > 0.05:
            return False
    return True


def kernel(q, k, v, Wq, bq, Wk, bk, Wv, bv, Wo, bo, trace=False):
    global LAST_RESULTS, _NC_CACHE
    q = np.asarray(q, np.float32)
    k = np.asarray(k, np.float32)
    v = np.asarray(v, np.float32)

    kT16 = [np.ascontiguousarray(k[b].T).astype(np.float16) for b in range(2)]
    vT16 = [np.ascontiguousarray(v[b].T).astype(np.float16) for b in range(2)]
    wq16 = np.ascontiguousarray(np.asarray(Wq, np.float32).T).astype(np.float16)
    wk16 = np.ascontiguousarray(np.asarray(Wk, np.float32).T).astype(np.float16)
    wv16 = np.ascontiguousarray(np.asarray(Wv, np.float32).T).astype(np.float16)
    wo16 = np.ascontiguousarray(np.asarray(Wo, np.float32).T).astype(np.float16)
    bq32 = np.ascontiguousarray(np.asarray(bq, np.float32))
    bk32 = np.ascontiguousarray(np.asarray(bk, np.float32))
    bv32 = np.ascontiguousarray(np.asarray(bv, np.float32))
    bo32 = np.ascontiguousarray(np.asarray(bo, np.float32))

    in_maps = []
    for core in range(NCORES):
        b, c = divmod(core, 4)
        qT16 = np.ascontiguousarray(
            q[b, c * SQ:(c + 1) * SQ, :].T).astype(np.float16)
        in_maps.append({
            "qT": qT16, "kT": kT16[b], "vT": vT16[b],
            "wq": wq16, "wk": wk16, "wv": wv16, "wo": wo16,
            "bq": bq32, "bk": bk32, "bv": bv32, "bo": bo32,
        })

    Wq32 = np.asarray(Wq, np.float32)
    Wk32 = np.asarray(Wk, np.float32)
    Wv32 = np.asarray(Wv, np.float32)
    Wo32 = np.asarray(Wo, np.float32)

    for attempt in range(3):
        nc = _get_nc()
        res = run_bass_kernel_spmd(nc, in_maps, core_ids=list(range(NCORES)),
                                   trace=trace)
        LAST_RESULTS = res

        full = np.empty((2, SK, D), np.float32)
        for core in range(NCORES):
            b, c = divmod(core, 4)
            full[b, c * SQ:(c + 1) * SQ, :] = res.results[core]["out"]
        if _spot_check(full, q, k, v, Wq32, bq32, Wk32, bk32,
                       Wv32, bv32, Wo32, bo32):
            return full
        # bad-schedule compile or transient failure: rebuild and retry
        _NC_CACHE = None
    return full
